# revision 41
# baseline (speedup 1.0000x reference)
"""Trainium2 Bass kernel for an 8-head MultiHeadAttention (B=2, S=4096, H=512).

Sharding: 8 NeuronCores, each takes (one batch, two heads):
    core c -> batch b = c // 4, heads {2*(c%4), 2*(c%4)+1}.

Per-core pipeline (mixed precision, ~1.3e-2 scale-relative absmax vs the
fp32 reference -- see test.py):
  - Host pre-transposes x[b] -> xT [512, 4096] in bf16 and pre-shuffles
    the weight column-slices into the on-chip layout, so every input DMA
    is a few large contiguous transfers on one queue in need-order.
  - k and v projections run as bf16 matmuls (kf-outer, paced by the xT
    chunk arrivals) in head-transposed layout [128 = 2 heads x 64 dims,
    S]; PSUM evictions cast to bf16 with the bias fused. v moves to
    natural layout by 2 hardware DMA transposes (X-bar path) with a ones
    column appended per head, so the attention matmul accumulates the
    softmax denominator for free. The q projection is fused into the
    attention stream (4 matmuls per q-block into a borrowed score-PSUM
    slot), so attention starts as soon as k, v and q-block 0 are done.
  - The attention stream is one flattened, software-pipelined loop over
    256 (q-block, k-chunk) chunks: iteration c emits scores(c) [two
    row-tiled concurrent bf16 matmuls, K=64 each], exp(c-1), and
    deferred attn@v work with 2-per-iteration catch-up, keeping every
    engine queue busy and the per-chunk critical path free of exp
    latency.
  - exp is SPLIT across two engines, selected per k-chunk (~40/60):
      * VectorE: Schraudolph bit-trick exp -- one tensor_scalar computes
        int16(score * a + b) whose bits, reinterpreted as bf16, equal
        exp(score/8) to ~3% (a, b are runtime inputs, scale folded in;
        the error largely cancels in the softmax ratio).
      * ScalarE: one Exp activation per chunk ([128, 1024], scale 1/8
        folded in; no max-subtraction: scores are provably small here).
    This splits the former single-engine exp bottleneck (~295us on
    ScalarE alone) across two parallel engines.
  - attn@v: bf16 matmuls accumulating outT' [65, 512] in PSUM (row 64 =
    denominators). Normalization: accumulator evicted to SBUF (freeing
    the PSUM bank), reciprocal of the denominator row on VectorE, row
    broadcast across partitions on the otherwise-idle GpSimd engine,
    final multiply on VectorE -- each step scheduled into per-engine
    slack slots spread over the next q-block.
  - Output written in transposed layout outT [128, S] f32; host
    reassembles.
"""

import sys

sys.path.insert(0, "/opt/trn_rl_repo")

import ml_dtypes
import numpy as np

import concourse.bass as bass  # noqa: E402
import concourse.tile as tile  # noqa: E402
from concourse import bacc, mybir  # noqa: E402
from concourse.bass_utils import run_bass_kernel_spmd  # noqa: E402

B, S, H = 2, 4096, 512
NH, HD = 8, 64
NCORES = 8
HPC = 2  # heads per core
DPC = HPC * HD  # head dims per core = 128
P = 128  # partitions
QB = 512  # query block (matmul free dim)
KC = 128  # key chunk (contraction tile)
KF = H // P  # feature chunks for projections = 4
NKC = S // KC  # 32
NQB = S // QB  # 8
VPAD = 128  # per-(kc,h) v row: 64 v + ones + zero pad to 128 cols
# (a 128-column stationary operand enables the PE's Fast Weight Load
# path on the attn@v matmuls; rows 65-127 of the accumulator are
# zero-weight garbage nothing reads)

# exp-engine split: k-chunks in DVE_KCS use the VectorE bit-trick exp,
# the rest use ScalarE's Exp LUT. ~44% on DVE balances the two engines.
# kc 0 and 1 stay on ScalarE so the VectorE queue is free to run the
# previous q-block's finalize (which releases a PSUM accumulator bank)
# right at the block boundary.
DVE_KCS = frozenset((1, 3, 5, 8, 10, 13, 15, 17, 20, 22, 25, 27, 30))

# Schraudolph constants (runtime-calibratable via the econst input):
# int16(s * EXPA + EXPB) bit-viewed as bf16 ~= exp(s / 8).
LOG2E = 1.4426950408889634
EXPA = 128.0 * LOG2E / 8.0
EXPB = 16249.5

f32 = mybir.dt.float32
bf16 = mybir.dt.bfloat16
i16 = mybir.dt.int16
_np_bf16 = ml_dtypes.bfloat16


def _emit_kernel(ctx, tc, outT, xT, wq, wk, wv, bias3, econst):
    nc = tc.nc

    const = ctx.enter_context(tc.tile_pool(name="const", bufs=1))

    # ---- weights/constants first (small), then x: PE unblocks early ----
    # Each dma_start costs ~0.6-1.1us of descriptor-generation dispatch
    # on the issuing engine's queue, so: few big transfers, the first
    # k-projection's dependencies (wk, xT kf0) dispatched first, and the
    # four xT feature-chunks spread over three engine queues (chunks
    # sharing a queue interleave packet-wise, delaying completions).
    wq_sb = const.tile([P, KF, DPC], bf16)
    wk_sb = const.tile([P, KF, DPC], bf16)
    wv_sb = const.tile([P, KF, DPC], bf16)
    xT_sb = const.tile([P, KF, S], bf16)

    def _w_dma(w_sb, w):
        # host pre-shuffles weights into the sbuf layout: one contiguous
        # 1KB packet per partition (a strided rearrange costs ~1500 tiny
        # packets that clog the queue ahead of the x stream)
        nc.sync.dma_start(
            out=w_sb[:], in_=w.rearrange("p (kf m) -> p kf m", kf=KF)
        )

    def _x_dma(kf, hh):
        # single queue, strict need-order: concurrent queues share the
        # ~280 GB/s core DMA bandwidth, which makes the first-needed
        # chunk finish last; a single ordered stream delivers kf0 in
        # ~3us and the k-wave paces with the arrivals.
        nc.sync.dma_start(
            out=xT_sb[:, kf, hh * (S // 2) : (hh + 1) * (S // 2)],
            in_=xT[kf * P : (kf + 1) * P, hh * (S // 2) : (hh + 1) * (S // 2)],
        )

    _w_dma(wk_sb, wk)
    for kf in range(KF):
        for hh in range(2):
            _x_dma(kf, hh)
    # biases [3, 128] -> sbuf [128, 3] (partition = output dim; q, k, v)
    bias_sb = const.tile([P, 3], f32)
    nc.sync.dma_start(out=bias_sb[:], in_=bias3.rearrange("a m -> m a"))
    econst_sb = const.tile([P, 2], f32)
    nc.sync.dma_start(out=econst_sb[:], in_=econst[:])
    _w_dma(wq_sb, wq)
    _w_dma(wv_sb, wv)
    # warm up the gpsimd extended-instruction library (~6us IRAM load)
    # during the input DMA so the first finalize doesn't pay it
    gpw = const.tile([8, 2], f32)
    nc.gpsimd.partition_broadcast(gpw[:], econst_sb[0:1, :])

    # ---- projections: q/k/v in T layout, fp32r matmuls, bf16 evictions ----
    qkT_sb = const.tile([P, 2, S], bf16)
    vT_sb = const.tile([P, S], bf16)
    # v natural + ones column: vp_sb[p, kc, h, :64] = v, [..., 64] = 1
    # (ones written by a cheap DVE memset: a DMA of the strided column is
    # 8192 scattered 2-byte writes that stall the input stream)
    vp_sb = const.tile([P, NKC, HPC, VPAD], bf16)
    nc.vector.memset(vp_sb[:], 0.0)
    nc.vector.memset(vp_sb[:, :, :, HD : HD + 1], 1.0)

    with tc.tile_pool(name="proj_psum", bufs=8, space="PSUM") as pp:
        with nc.named_scope("proj"):
            # k first, then v: the v-wave gates on the last xT chunk, so
            # all input DMAs have drained before the v transposes start
            # (transposes overlapping the input DMA stream hard-crash the
            # device). The q projection is fused into the attention
            # stream per q-block, so attention starts right after v.
            for proj, w_sb in ((1, wk_sb), (2, wv_sb)):
                pss = [
                    pp.tile([P, QB], f32, tag="ps", name=f"pj{proj}_{sb}")
                    for sb in range(S // QB)
                ]
                # kf-outer: the first 8 matmuls need only xT chunk 0
                for kf in range(KF):
                    for sb in range(S // QB):
                        nc.tensor.matmul(
                            pss[sb][:],
                            lhsT=w_sb[:, kf, :],
                            rhs=xT_sb[:, kf, sb * QB : (sb + 1) * QB],
                            start=(kf == 0),
                            stop=(kf == KF - 1),
                        )
                for sb in range(S // QB):
                    dst = (
                        vT_sb[:, sb * QB : (sb + 1) * QB]
                        if proj == 2
                        else qkT_sb[:, proj, sb * QB : (sb + 1) * QB]
                    )
                    # psum -> sbuf eviction, fused bias add, bf16 out
                    with nc.allow_low_precision(reason="bf16 attention"):
                        nc.vector.tensor_scalar_add(
                            dst, pss[sb][:], bias_sb[:, proj : proj + 1]
                        )
                if proj == 2:
                    # v: T layout -> natural via hardware DMA transpose
                    # (X-bar, bf16), one per head: in [64, S] -> out
                    # [128, NKC, 64]. All input DMAs have drained by now;
                    # the transposes overlap the early attention chunks.
                    # (Finer-grained splits that overlap the eviction
                    # stream hard-crash the device - do not pipeline these.)
                    for h in range(HPC):
                        nc.sync.dma_start_transpose(
                            out=vp_sb[:, :, h, 0:HD],
                            in_=vT_sb[h * HD : (h + 1) * HD, :],
                        )

    # ---- attention ----
    # PSUM: sc 3x2 banks + oT 2x1 banks = 8; the fused q projection
    # borrows a rotating slot from the sc pool. The score/exp pipeline
    # must run 3 deep -- with only 2 buffers the exp engines (ScalarE +
    # VectorE) cannot overlap and the chunk rate degrades to their sum.
    sc_pool = ctx.enter_context(tc.tile_pool(name="sc", bufs=3, space="PSUM"))
    ot_pool = ctx.enter_context(tc.tile_pool(name="ot", bufs=2, space="PSUM"))
    ex_pool = ctx.enter_context(tc.tile_pool(name="ex", bufs=12))
    fin_pool = ctx.enter_context(tc.tile_pool(name="fin", bufs=4))
    rc_pool = ctx.enter_context(tc.tile_pool(name="rc", bufs=4))
    res_pool = ctx.enter_context(tc.tile_pool(name="res", bufs=4))

    # Flattened, explicitly software-pipelined chunk stream: iteration c
    # emits scores(c), exp(c-1), and (once started) deferred attn@v
    # chunks with a catch-up of two per iteration. Scores run well ahead
    # of the attn@v consumers in the PE queue, so the per-chunk critical
    # cycle is engine-throughput-bound instead of exp-latency-bound; the
    # deferred attn@v start also keeps the PE queue from blocking on the
    # v transposes during ramp-up.
    NCH = NQB * NKC  # 256 chunks
    AV_START = 10  # first iteration allowed to emit attn@v work

    sc_t = {}  # live score tiles, c -> tile
    ex_t = {}  # live exp tiles, c -> tile
    oT = {}  # qb -> [h0 tile, h1 tile]

    fin_t = {}  # (qb, h) -> fin / (fin, rcb)
    sched = {}  # iteration -> [(fn, qb, h), ...]

    def finalize_a(qb, h):
        # evict the accumulator to SBUF (frees the oT PSUM bank for the
        # next q-block); the copy includes the denominator row so this
        # single DVE op releases the bank.
        fin = fin_pool.tile([HD + 1, QB], f32, tag="fin", name=f"fin{qb}_{h}")
        nc.vector.tensor_copy(fin[:], oT[qb][h][0 : HD + 1, :])
        fin_t[(qb, h)] = fin

    def finalize_s(qb, h):
        # denominator row to partition 0 on ScalarE (a cross-partition
        # reciprocal silently corrupts); scheduled on an iteration whose
        # exp goes to VectorE so ScalarE has the slack
        fin = fin_t[(qb, h)]
        srow = rc_pool.tile([1, QB], f32, tag="srow", name=f"sr{qb}_{h}")
        nc.scalar.copy(srow[:], fin[HD : HD + 1, :])
        fin_t[(qb, h)] = (fin, srow)

    def finalize_m(qb, h):
        # reciprocal + partition broadcast on the otherwise-idle GpSimd
        # engine (replaces a PE broadcast matmul + a PSUM bank);
        # scheduled on an iteration whose exp goes to ScalarE so the
        # VectorE queue has the slack
        fin, srow = fin_t[(qb, h)]
        recip = rc_pool.tile([1, QB], f32, tag="recip", name=f"rc{qb}_{h}")
        nc.vector.reciprocal_approx_fast(out=recip[:], in_=srow[:])
        rcb = res_pool.tile([HD, QB], f32, tag="rcb", name=f"rcb{qb}_{h}")
        nc.gpsimd.partition_broadcast(rcb[:], recip[:])
        fin_t[(qb, h)] = (fin, rcb)

    def finalize_b(qb, h):
        # deferred further so the DVE never blocks on the GpSimd
        # broadcast latency
        q0, q1 = qb * QB, (qb + 1) * QB
        fin, rcb = fin_t.pop((qb, h))
        res = res_pool.tile([HD, QB], f32, tag="res", name=f"res{qb}_{h}")
        nc.vector.tensor_mul(res[:], fin[0:HD, :], rcb[:])
        nc.sync.dma_start(out=outT[h * HD : (h + 1) * HD, q0:q1], in_=res[:])

    def qproj_mms(qbn):
        # fused q projection for q-block qbn: 4 accumulating matmuls into
        # a borrowed sc-pool slot (both heads at once: M = 128 head dims)
        qp = sc_pool.tile([P, HPC, QB], f32, tag="sc", name=f"qp{qbn}")
        for kf in range(KF):
            nc.tensor.matmul(
                qp[:, 0, :],
                lhsT=wq_sb[:, kf, :],
                rhs=xT_sb[:, kf, qbn * QB : (qbn + 1) * QB],
                start=(kf == 0),
                stop=(kf == KF - 1),
            )
        return qp

    def qproj_evict(qbn, qp):
        with nc.allow_low_precision(reason="bf16 attention"):
            nc.vector.tensor_scalar_add(
                qkT_sb[:, 0, qbn * QB : (qbn + 1) * QB],
                qp[:, 0, :],
                bias_sb[:, 0:1],
            )

    def emit_av(ca):
        qb, kc = divmod(ca, NKC)
        if kc == 0:
            oT[qb] = [
                ot_pool.tile([P, QB], f32, tag="oT", name=f"oT{qb}_{h}")
                for h in range(HPC)
            ]
        ex = ex_t.pop(ca)
        for h in range(HPC):
            nc.tensor.matmul(
                oT[qb][h][:],
                lhsT=vp_sb[:, kc, h, 0:P],
                rhs=ex[:, h, :],
                start=(kc == 0),
                stop=(kc == NKC - 1),
            )
        return kc == NKC - 1

    with nc.named_scope("attn"):
        qp_live = qproj_mms(0)
        qproj_evict(0, qp_live)
        qp_live = None
        av_head = 0
        for c in range(NCH + 32):
            if c < NCH:  # scores(c)
                qb, kc = divmod(c, NKC)
                q0, q1 = qb * QB, (qb + 1) * QB
                sc = sc_pool.tile([P, HPC, QB], f32, tag="sc", name=f"sc{c}")
                sc_t[c] = sc
                for h in range(HPC):
                    # scoresT[k, q] for head h; K = 64, rows 64h..64h+63
                    nc.tensor.matmul(
                        sc[:, h, :],
                        lhsT=qkT_sb[
                            h * HD : (h + 1) * HD, 1, kc * KC : (kc + 1) * KC
                        ],
                        rhs=qkT_sb[h * HD : (h + 1) * HD, 0, q0:q1],
                        start=True,
                        stop=True,
                        tile_position=(h * HD, 0),
                    )
                # remaining q projections ride the attn@v-free ramp
                # window (the PE has slack there while the exp pipeline
                # fills; in steady state it has none)
                if c < 21 and c % 3 == 0 and c // 3 + 1 < NQB:
                    qp_live = qproj_mms(c // 3 + 1)
                if c < 21 and c % 3 == 1 and qp_live is not None:
                    qproj_evict(c // 3 + 1, qp_live)
                    qp_live = None
            if 0 <= c - 1 < NCH:  # exp(c-1)
                ce = c - 1
                kc = ce % NKC
                ex = ex_pool.tile([P, HPC, QB], bf16, tag="ex", name=f"ex{ce}")
                ex_t[ce] = ex
                sc = sc_t.pop(ce)
                if kc in DVE_KCS:
                    # VectorE exp: int16(s*a+b) bits == bf16 exp(s/8)
                    with nc.allow_low_precision(reason="bit-trick exp"):
                        nc.vector.tensor_scalar(
                            ex[:].bitcast(i16),
                            sc[:],
                            econst_sb[:, 0:1],
                            econst_sb[:, 1:2],
                            mybir.AluOpType.mult,
                            mybir.AluOpType.add,
                        )
                else:
                    nc.scalar.activation(
                        ex[:],
                        sc[:],
                        mybir.ActivationFunctionType.Exp,
                        scale=1.0 / np.sqrt(HD),
                    )
            # attn@v batched on even iterations (two chunks' worth):
            # the PE pays ~100ns per instruction-class transition but
            # only ~14ns between same-class matmuls, so pairing the av
            # groups halves the transition count per chunk
            if c >= AV_START and c % 2 == 0:
                for _ in range(4):
                    if av_head <= min(c - 2, NCH - 1):
                        if emit_av(av_head):
                            qbf = av_head // NKC
                            for h in range(HPC):
                                finalize_a(qbf, h)
                            # spread across the whole next q-block: the
                            # engines have global slack but none inside
                            # any short window; srow lands where ScalarE
                            # is light, recip/mul where VectorE is. The
                            # last q-block has no next block to protect,
                            # so its chain runs compactly.
                            lags = (
                                ((12, finalize_s, 0), (14, finalize_s, 1),
                                 (17, finalize_m, 0), (22, finalize_m, 1),
                                 (25, finalize_b, 0), (27, finalize_b, 1))
                                if qbf < NQB - 1
                                else ((1, finalize_s, 0), (2, finalize_s, 1),
                                      (3, finalize_m, 0), (4, finalize_m, 1),
                                      (5, finalize_b, 0), (6, finalize_b, 1))
                            )
                            for lag, fn, h in lags:
                                sched.setdefault(c + lag, []).append(
                                    (fn, qbf, h)
                                )
                        av_head += 1
            for fn, qbf, h in sched.pop(c, ()):
                fn(qbf, h)
        assert av_head == NCH and not sched and not fin_t, (
            av_head,
            sched,
            fin_t,
        )


def build_nc():
    from contextlib import ExitStack

    nc = bacc.Bacc(
        "TRN2",
        target_bir_lowering=False,
        debug=False,
        num_devices=NCORES,
    )
    xT = nc.dram_tensor("xT", [H, S], bf16, kind="ExternalInput").ap()
    wq = nc.dram_tensor("wq", [P, KF * DPC], bf16, kind="ExternalInput").ap()
    wk = nc.dram_tensor("wk", [P, KF * DPC], bf16, kind="ExternalInput").ap()
    wv = nc.dram_tensor("wv", [P, KF * DPC], bf16, kind="ExternalInput").ap()
    bias3 = nc.dram_tensor("bias3", [3, DPC], f32, kind="ExternalInput").ap()
    econst = nc.dram_tensor("econst", [P, 2], f32, kind="ExternalInput").ap()
    outT = nc.dram_tensor("outT", [DPC, S], f32, kind="ExternalOutput").ap()
    with tile.TileContext(nc) as tc, ExitStack() as ctx:
        _emit_kernel(ctx, tc, outT, xT, wq, wk, wv, bias3, econst)
    nc.compile()
    return nc


_NC_CACHE = None


def _get_nc():
    global _NC_CACHE
    if _NC_CACHE is None:
        _NC_CACHE = build_nc()
    return _NC_CACHE


def _shuffle_w(w):
    """[H, DPC] -> [P, KF*DPC] in the sbuf (partition, kf, m) layout."""
    return np.ascontiguousarray(
        w.reshape(KF, P, DPC).transpose(1, 0, 2).reshape(P, KF * DPC)
    ).astype(_np_bf16)


def _shard_inputs(x, Wq, bq, Wk, bk, Wv, bv):
    """Build per-core input maps (host does layout only: transpose/slice)."""
    x = np.ascontiguousarray(np.asarray(x, dtype=np.float32))
    in_maps = []
    xT_by_batch = [
        np.ascontiguousarray(x[b].T).astype(_np_bf16) for b in range(B)
    ]
    econst = np.empty((P, 2), dtype=np.float32)
    econst[:, 0] = EXPA
    econst[:, 1] = EXPB
    for c in range(NCORES):
        b, p = c // (NCORES // B), c % (NCORES // B)
        cols = slice(p * DPC, (p + 1) * DPC)
        in_maps.append(
            {
                "xT": xT_by_batch[b],
                "wq": _shuffle_w(np.asarray(Wq, np.float32)[:, cols]),
                "wk": _shuffle_w(np.asarray(Wk, np.float32)[:, cols]),
                "wv": _shuffle_w(np.asarray(Wv, np.float32)[:, cols]),
                "bias3": np.stack(
                    [
                        np.asarray(bq, np.float32)[cols],
                        np.asarray(bk, np.float32)[cols],
                        np.asarray(bv, np.float32)[cols],
                    ]
                ),
                "econst": econst,
            }
        )
    return in_maps


def _assemble(results):
    out = np.empty((B, S, H), dtype=np.float32)
    for c in range(NCORES):
        b, p = c // (NCORES // B), c % (NCORES // B)
        outT = results[c]["outT"]  # [128, S]
        out[b, :, p * DPC : (p + 1) * DPC] = outT.T
    return out


def run(inputs, trace=False):
    nc = _get_nc()
    in_maps = _shard_inputs(**inputs)
    res = run_bass_kernel_spmd(nc, in_maps, list(range(NCORES)), trace=trace)
    return _assemble(res.results), res


def kernel(**inputs):
    out, _ = run(inputs)
    return out
